# revision 38
# baseline (speedup 1.0000x reference)
"""Trainium2 Bass kernel for nn_Bert_10187662426159 (DeBERTa-style
disentangled-attention BERT layer, L=512 B=16 D=1024 H=16).

Sharding: data-parallel over B — core c handles batch entries {2c, 2c+1}.

Per-core pipeline (ST orientation: scores stored [key j on partitions,
query i on free dim]; all matmul operands f16, PSUM accumulation f32):
  P1  LN1 (no affine) -> h ; PE-transpose -> hT [feat, tok]
  P1b q/k proj (feat-major, q pre-scaled by 1/sqrt(3*64)), v proj
      (token-major, with a ones-column per head for softmax row sums),
      rel-pos proj qkposT [2048, 63]
  P2  per group of 4 heads:
        qpST/pkST [63, 512] rank-63 positional factors (per head)
        window expansion [128, 640] per 128-row tile via 0/1 G-matrices
        (raw scores in delta space, f16 directly in PSUM)
        skew via DRAM bounce: ONE contiguous write of the whole group's
        windows [128, 32 slots, 640] to a DRAM scratch, then ONE read
        back whose partition stride is (32*640 - 1): partition p lands
        at flat offset p*(32*640-1)+127+..., i.e. the exact per-row
        shift 127-p that realigns delta-space diagonals to absolute
        key positions. 2 big DMAs replace 48 small strided ones.
      per head, per 128-row j-tile, scores assemble in one PSUM bank:
        c2c matmul + pk via identity-matmul accumulate + qp via four
        identity-rhs transpose-matmul accumulates; ONE ACT exp with the
        attention mask as a per-partition bias (-1e9) -> P (f16, no
        max-subtraction needed: scores are bounded, exp(-1e9) = 0)
        ctx: [v | 1]^T @ P accumulates context AND row sums in PSUM;
        1/sum broadcast to 64 partitions via a ones-outer-product
        matmul into PSUM; DVE multiply normalizes
  P3  y = ctxT^T @ woT, LN2 + affine.

The DeBERTa take_along_axis gathers are exact: bucket expansion is a 0/1
matmul in delta-space and the diagonal re-alignment (skew) is one exact
DRAM round-trip (per-partition offsets are outside SBUF AP
expressiveness, but DRAM is flat-addressed, so a read AP with partition
stride (slot_stride - 1) applies a per-partition shift of -p exactly).
"""
import contextlib
import math
import sys

import numpy as np

sys.path.insert(0, "/opt/trn_rl_repo")
sys.path.insert(0, "/opt/trn_rl_repo/concourse")

import concourse.mybir as mybir  # noqa: E402
import concourse.tile as tile  # noqa: E402
from concourse import bacc, bass, bass_utils  # noqa: E402
from concourse.masks import make_identity  # noqa: E402

F32 = mybir.dt.float32
F16 = mybir.dt.float16
F8 = mybir.dt.float8e4

HIDDEN, HEADS, HEAD = 1024, 16, 64
BUCKET, MAXPOS, REL = 32, 512, 63
L, B = 512, 16
EPS = 1e-7
SCALE = 1.0 / math.sqrt(3 * HEAD)
WIN = 640
NCORES = 8
BLOC = B // NCORES          # 2 batch entries per core
NTOK = L * BLOC             # 1024 tokens per core
NT = NTOK // 128            # 8 token tiles
GH = 2                      # heads per attention group
SLOTS = 2 * GH * 4          # (side, head, i-tile) windows per group
AF = mybir.ActivationFunctionType


def _bucket_fn(delta):
    r = np.asarray(delta)
    mid = BUCKET // 2
    abs_pos = np.where((r < mid) & (r > -mid), mid - 1,
                       np.minimum(np.abs(r), MAXPOS - 1))
    with np.errstate(divide="ignore"):
        log_pos = (np.ceil(np.log(abs_pos.astype(np.float64) / mid)
                           / math.log((MAXPOS - 1) / mid) * (mid - 1))
                   .astype(np.int64) + mid)
    bucket_pos = np.where(abs_pos <= mid, r, log_pos * np.sign(r))
    return (BUCKET - 1 + bucket_pos).astype(np.int64)


def _make_G():
    Gq, Gk = [], []
    for t in range(4):
        w0 = -127 - 128 * t
        c = np.arange(WIN)
        dq = np.clip(-(w0 + c), -511, 511)
        dk = np.clip(+(w0 + c), -511, 511)
        Gq.append(_bucket_fn(dq)[None, :] == np.arange(REL)[:, None])
        Gk.append(_bucket_fn(dk)[None, :] == np.arange(REL)[:, None])
    return (np.stack(Gq).transpose(1, 0, 2).astype(np.float16),
            np.stack(Gk).transpose(1, 0, 2).astype(np.float16))  # [63, 4, 640]


def _build(with_bias: bool, with_affine: bool):
    nc = bacc.Bacc("TRN2", debug=False, num_devices=NCORES)

    hs_d = nc.dram_tensor("hs_tok", (NTOK, HIDDEN), F16, kind="ExternalInput").ap()
    mb_d = nc.dram_tensor("maskbias", (128, BLOC * 4), F32, kind="ExternalInput").ap()
    wqkT_d = nc.dram_tensor("wqkT", (HIDDEN, 2 * HIDDEN), F16, kind="ExternalInput").ap()
    wvT_d = nc.dram_tensor("wvT", (HIDDEN, HIDDEN), F16, kind="ExternalInput").ap()
    woT_d = nc.dram_tensor("woT", (HIDDEN, HIDDEN), F16, kind="ExternalInput").ap()
    relT_d = nc.dram_tensor("relT", (HIDDEN, 64), F16, kind="ExternalInput").ap()
    gq_d = nc.dram_tensor("Gq", (REL, 4, WIN), F16, kind="ExternalInput").ap()
    gk_d = nc.dram_tensor("Gk", (REL, 4, WIN), F16, kind="ExternalInput").ap()
    if with_bias:
        bqk_d = nc.dram_tensor("bqk2", (1, 2 * HIDDEN), F16, kind="ExternalInput").ap()
        bv_d = nc.dram_tensor("bv2", (1, HIDDEN), F16, kind="ExternalInput").ap()
        ones_d = nc.dram_tensor("ones_row", (1, NTOK), F16, kind="ExternalInput").ap()
    if with_affine:
        g_d = nc.dram_tensor("g_bcast", (128, HIDDEN), F32, kind="ExternalInput").ap()
        b_d = nc.dram_tensor("b_bcast", (128, HIDDEN), F32, kind="ExternalInput").ap()
    out_d = nc.dram_tensor("out_y", (NTOK, HIDDEN), F32, kind="ExternalOutput").ap()
    # DRAM scratch for the skew bounce: one buffer per (group parity, head)
    # so no two in-flight half-bounces share a buffer (the diagonal read AP
    # spans the whole buffer byte-range, which would serialize them).
    skw_h = [nc.dram_tensor(f"skw{i}", (128, 8, WIN), F16, kind="Internal")
             for i in range(4)]

    with tile.TileContext(nc) as tc, contextlib.ExitStack() as ctx:
        consts = ctx.enter_context(tc.tile_pool(name="consts", bufs=1))
        wpool = ctx.enter_context(tc.tile_pool(name="wpool", bufs=2))
        xio = ctx.enter_context(tc.tile_pool(name="xio", bufs=2))
        stat = ctx.enter_context(tc.tile_pool(name="stat", bufs=4))
        big = ctx.enter_context(tc.tile_pool(name="big", bufs=1))
        atts = ctx.enter_context(tc.tile_pool(name="atts", bufs=2))
        attp = ctx.enter_context(tc.tile_pool(name="attp", bufs=2))
        # PSUM (8 banks): b1 = 3 x 1 bank (assembly/proj), w5 = 3 x 1 bank
        # (factors + 512-wide expansion), w1 = 2 x 1 bank (128-wide tail).
        psp = ctx.enter_context(tc.tile_pool(name="psp", bufs=3, space="PSUM"))
        pse = ctx.enter_context(tc.tile_pool(name="pse", bufs=3, space="PSUM"))
        ps1 = ctx.enter_context(tc.tile_pool(name="ps1", bufs=2, space="PSUM"))

        # ---------- constants ----------
        ident16 = consts.tile([128, 128], F16)
        make_identity(nc, ident16)

        eps_t = consts.tile([128, 1], F32)
        nc.vector.memset(eps_t, EPS)
        gq_s = consts.tile([REL, 4, WIN], F16)
        gk_s = consts.tile([REL, 4, WIN], F16)
        nc.gpsimd.dma_start(out=gq_s, in_=gq_d)
        nc.gpsimd.dma_start(out=gk_s, in_=gk_d)
        mb_s = consts.tile([128, BLOC * 4], F32)
        nc.gpsimd.dma_start(out=mb_s, in_=mb_d)
        relT_s = consts.tile([128, NT, 64], F16)
        nc.gpsimd.dma_start(out=relT_s, in_=relT_d.rearrange("(n p) r -> p n r", p=128))
        onecol = consts.tile([1, 64], F16)
        nc.vector.memset(onecol, 1.0)
        # touch Exp early so the activation-table load doesn't stall the
        # first softmax at the P1b->P2 boundary
        warm = consts.tile([1, 1], F32)
        nc.scalar.activation(out=warm, in_=eps_t[0:1, :], func=AF.Exp)
        if with_bias:
            bqk_s = consts.tile([1, 2 * HIDDEN], F16)
            bv_s = consts.tile([1, HIDDEN], F16)
            ones_s = consts.tile([1, NTOK], F16)
            nc.sync.dma_start(out=bqk_s, in_=bqk_d)
            nc.sync.dma_start(out=bv_s, in_=bv_d)
            nc.sync.dma_start(out=ones_s, in_=ones_d)
        if with_affine:
            g_s = consts.tile([128, HIDDEN], F32)
            b_s = consts.tile([128, HIDDEN], F32)
            nc.sync.dma_start(out=g_s, in_=g_d)
            nc.sync.dma_start(out=b_s, in_=b_d)

        def layernorm_stats(y):
            """-> (rstd, -mean*rstd) [128,1] tiles for ACT normalize."""
            st = stat.tile([128, 2, nc.vector.BN_STATS_DIM], F32, tag="st")
            mv = stat.tile([128, nc.vector.BN_AGGR_DIM], F32, tag="mv")
            yr = y.rearrange("p (s d) -> p s d", s=2)
            for s in range(2):
                nc.vector.bn_stats(out=st[:, s, :], in_=yr[:, s, :])
            nc.vector.bn_aggr(out=mv, in_=st)
            rstd = stat.tile([128, 1], F32, tag="rstd")
            nc.scalar.activation(out=rstd, in_=mv[:, 1:2], func=AF.Sqrt,
                                 bias=eps_t, scale=1.0)
            nc.vector.reciprocal(out=rstd, in_=rstd)
            nmr = stat.tile([128, 1], F32, tag="nmr")
            nc.vector.tensor_mul(nmr, mv[:, 0:1], rstd)
            nc.vector.tensor_scalar_mul(nmr, nmr, -1.0)
            return rstd, nmr

        # ---------- P1: LN1 + transpose ----------
        hT = big.tile([128, NT, NTOK], F16, tag="hT")  # [feat, tok]
        hs3 = hs_d.rearrange("(n p) d -> n p d", p=128)
        for tt in range(NT):
            x = xio.tile([128, HIDDEN], F16, tag="xh")
            (nc.sync if tt % 2 == 0 else nc.scalar).dma_start(out=x, in_=hs3[tt])
            rstd, nmr = layernorm_stats(x)
            h = xio.tile([128, HIDDEN], F16, tag="hyo")
            nc.vector.tensor_scalar(out=h, in0=x, scalar1=rstd, scalar2=nmr,
                                    op0=mybir.AluOpType.mult,
                                    op1=mybir.AluOpType.add)
            for fq in range(2):
                ptr = psp.tile([128, 4, 128], F16, tag="b1")
                for fb in range(4):
                    nc.tensor.matmul(ptr[:, fb, :],
                                     h[:, 512 * fq + 128 * fb:512 * fq + 128 * fb + 128],
                                     ident16, is_transpose=True)
                if fq == 0:
                    nc.vector.tensor_copy(
                        out=hT[:, 4 * fq:4 * fq + 4, 128 * tt:128 * tt + 128], in_=ptr)
                else:
                    nc.scalar.copy(
                        out=hT[:, 4 * fq:4 * fq + 4, 128 * tt:128 * tt + 128], in_=ptr)

        # ---------- P1b/P2/P3 building blocks (emitted interleaved) ----------
        qT = big.tile([128, 8, NTOK], F16, tag="qT")
        kT = big.tile([128, 8, NTOK], F16, tag="kT")
        vtm = big.tile([128, NT, HEADS, HEAD + 1], F16, tag="v")
        nc.vector.memset(vtm[:, :, :, HEAD:HEAD + 1], 1.0)
        qkposT = big.tile([128, 16, REL], F16, tag="qkposT")
        wqk3 = wqkT_d.rearrange("(n p) m -> n p m", p=128)
        wv_s = big.tile([128, 8, HIDDEN], F16, tag="wvo")
        nc.sync.dma_start(out=wv_s, in_=wvT_d.rearrange("(n p) m -> p n m", p=128))
        ctx_t = [None, None]
        wo_s = None
        out3 = out_d.rearrange("(n p) d -> n p d", p=128)
        nmi = 0

        def proj_qk(mg):
            """q/k + rel-pos projection for one 128-wide output feature tile."""
            nonlocal nmi
            w_m = wpool.tile([128, 8, 128], F16, tag="wqk")
            (nc.sync if nmi % 2 == 0 else nc.scalar).dma_start(
                out=w_m,
                in_=wqk3[:, :, 128 * mg:128 * mg + 128].transpose([1, 0, 2]))
            nmi += 1
            for nn_ in range(2):
                ns = slice(512 * nn_, 512 * nn_ + 512)
                pq = psp.tile([128, 512], F32, tag="b1")
                for k in range(8):
                    nc.tensor.matmul(pq, w_m[:, k, :], hT[:, k, ns],
                                     start=(k == 0),
                                     stop=(k == 7 and not with_bias))
                if with_bias:
                    nc.tensor.matmul(pq, bqk_s[:, 128 * mg:128 * mg + 128],
                                     ones_s[:, ns], start=False, stop=True)
                dst = qT if mg < 8 else kT
                nc.vector.tensor_copy(out=dst[:, mg % 8, ns], in_=pq)
            pqp = psp.tile([128, 64], F32, tag="b1")
            for k in range(8):
                nc.tensor.matmul(pqp, w_m[:, k, :], relT_s[:, k, :],
                                 start=(k == 0), stop=(k == 7 and not with_bias))
            if with_bias:
                nc.tensor.matmul(pqp, bqk_s[:, 128 * mg:128 * mg + 128],
                                 onecol, start=False, stop=True)
            nc.vector.tensor_copy(out=qkposT[:, mg, :], in_=pqp[:, :REL])

        def proj_v(mt):
            """v projection (token-major, with ones column) for one token tile."""
            for nn_ in range(2):
                ns = slice(512 * nn_, 512 * nn_ + 512)
                pv = psp.tile([128, 512], F32, tag="b1")
                for k in range(8):
                    nc.tensor.matmul(pv, hT[:, k, 128 * mt:128 * mt + 128],
                                     wv_s[:, k, ns], start=(k == 0),
                                     stop=(k == 7 and not with_bias))
                if with_bias:
                    nc.tensor.matmul(pv, ones_s[:, 128 * mt:128 * mt + 128],
                                     bv_s[:, ns], start=False, stop=True)
                nc.vector.tensor_copy(
                    out=vtm[:, mt, 8 * nn_:8 * nn_ + 8, 0:HEAD],
                    in_=pv.rearrange("p (h d) -> p h d", d=HEAD))

        def p2_expand(bi, hg):
            """windows + skew bounce for heads [2hg, 2hg+1] of entry bi."""
            toks = slice(512 * bi, 512 * bi + 512)
            par = (bi * (HEADS // GH) + hg) % 2
            # Eqk[:, hi, side, t, :] = delta-space windows (f16)
            Eqk = atts.tile([128, GH, 2, 4, WIN], F16, tag="eqk")
            outQPK = atts.tile([128, GH, 2, 4, 512], F16, tag="oqpk")
            for hi in range(GH):
                hd = GH * hg + hi
                po = 64 * (hd % 2)
                pf = slice(po, po + 64)
                qTh = qT[pf, hd // 2, toks]
                kTh = kT[pf, hd // 2, toks]
                kposTh = qkposT[pf, 8 + hd // 2, :]
                qposTh = qkposT[pf, hd // 2, :]
                pqp = pse.tile([REL, 512], F32, tag="w5")
                nc.tensor.matmul(pqp, kposTh, qTh)
                qpST = attp.tile([REL, 512], F16, tag="qpST")
                nc.vector.tensor_copy(out=qpST, in_=pqp)
                pkp = pse.tile([REL, 512], F32, tag="w5")
                nc.tensor.matmul(pkp, qposTh, kTh)
                pkST = attp.tile([REL, 512], F16, tag="pkST")
                nc.vector.tensor_copy(out=pkST, in_=pkp)
                for t in range(4):
                    rs = slice(128 * t, 128 * t + 128)
                    p5q = pse.tile([128, 512], F32, tag="w5")
                    nc.tensor.matmul(p5q, qpST[:, rs], gq_s[:, t, :512])
                    p1q = ps1.tile([128, 128], F32, tag="w1")
                    nc.tensor.matmul(p1q, qpST[:, rs], gq_s[:, t, 512:])
                    p5k = pse.tile([128, 512], F32, tag="w5")
                    nc.tensor.matmul(p5k, pkST[:, rs], gk_s[:, t, :512])
                    p1k = ps1.tile([128, 128], F32, tag="w1")
                    nc.tensor.matmul(p1k, pkST[:, rs], gk_s[:, t, 512:])
                    nc.scalar.copy(out=Eqk[:, hi, 0, t, :512], in_=p5q)
                    nc.vector.tensor_copy(out=Eqk[:, hi, 0, t, 512:], in_=p1q)
                    nc.vector.tensor_copy(out=Eqk[:, hi, 1, t, :512], in_=p5k)
                    nc.scalar.copy(out=Eqk[:, hi, 1, t, 512:], in_=p1k)
                # skew half-bounce for this head: one contiguous write + one
                # diagonal-AP read. Read partition p starts at flat
                # p*(8*WIN-1) + 127, i.e. column (127 - p) of its own row:
                # the exact per-row shift that realigns diagonals.
                buf = skw_h[2 * par + hi]
                nc.sync.dma_start(out=buf.ap(), in_=Eqk[:, hi])
                diag = bass.AP(buf, 127,
                               [[8 * WIN - 1, 128], [WIN, 8], [1, 512]])
                nc.gpsimd.dma_start(out=outQPK[:, hi], in_=diag)
            return outQPK

        def p2_assemble(bi, hg, outQPK):
            """score assembly + softmax + context for one head group."""
            toks = slice(512 * bi, 512 * bi + 512)
            # per head: score assembly in PSUM, one exp, ctx
            for hi in range(GH):
                hd = GH * hg + hi
                po = 64 * (hd % 2)
                pf = slice(po, po + 64)
                qTh = qT[pf, hd // 2, toks]
                kTh = kT[pf, hd // 2, toks]
                pctx = psp.tile([65, 512], F32, tag="b1")
                for jt in range(4):
                    js = slice(128 * jt, 128 * jt + 128)
                    pst = psp.tile([128, 512], F32, tag="b1")
                    nc.tensor.matmul(pst, kTh[:, js], qTh,
                                     start=True, stop=False)
                    nc.tensor.matmul(pst, ident16, outQPK[:, hi, 1, jt, :],
                                     start=False, stop=False)
                    for it in range(4):
                        nc.tensor.matmul(
                            pst[:, 128 * it:128 * it + 128],
                            outQPK[:, hi, 0, it, js], ident16,
                            start=False, stop=(it == 3))
                    P = attp.tile([128, 512], F16, tag="P")
                    nc.scalar.activation(
                        out=P, in_=pst, func=AF.Exp,
                        bias=mb_s[:, 4 * bi + jt:4 * bi + jt + 1])
                    nc.tensor.matmul(pctx, vtm[:, 4 * bi + jt, hd, :],
                                     P, start=(jt == 0), stop=(jt == 3))
                rsum = attp.tile([1, 512], F16, tag="rsum")
                with nc.allow_low_precision(reason="1/softmax-sum in f16 is ample"):
                    nc.vector.reciprocal(out=rsum, in_=pctx[64:65, :])
                pbc = psp.tile([64, 512], F32, tag="b1")
                nc.tensor.matmul(pbc, onecol, rsum)
                rb = attp.tile([64, 512], F16, tag="P")
                nc.scalar.copy(out=rb, in_=pbc)
                nc.vector.tensor_mul(ctx_t[bi][pf, hd // 2, :], pctx[0:64, :], rb)

        def p3_tile(mt):
            """wo projection + LN2 for one 128-token output tile."""
            bi, mtb = mt // 4, mt % 4
            y = xio.tile([128, HIDDEN], F32, tag="xy")
            for nn_ in range(2):
                ns = slice(512 * nn_, 512 * nn_ + 512)
                py = pse.tile([128, 512], F32, tag="w5")
                for k in range(8):
                    nc.tensor.matmul(
                        py, ctx_t[bi][:, k, 128 * mtb:128 * mtb + 128],
                        wo_s[:, k, ns], start=(k == 0), stop=(k == 7))
                nc.scalar.copy(out=y[:, ns], in_=py)
            rstd, nmr = layernorm_stats(y)
            yo = xio.tile([128, HIDDEN], F32, tag="hyo")
            nc.vector.tensor_scalar(out=yo, in0=y, scalar1=rstd, scalar2=nmr,
                                    op0=mybir.AluOpType.mult,
                                    op1=mybir.AluOpType.add)
            if with_affine:
                nc.vector.tensor_mul(yo, yo, g_s)
                nc.vector.tensor_add(yo, yo, b_s)
            nc.sync.dma_start(out=out3[mt], in_=yo)

        # ---------- emission order: sequential phases, with the first
        # attention group's expansion pulled into the P1b tail so its skew
        # bounce hides under the projection matmuls.
        for mg in [v for p in range(8) for v in (p, p + 8)]:
            proj_qk(mg)
        for mt in range(6):
            proj_v(mt)
        oq0 = p2_expand(0, 0)
        for mt in range(6, NT):
            proj_v(mt)
        # wo prefetch: reuses the wv slot once the v projection drains;
        # context reuses the hT slot (all hT readers are emitted above).
        wo_s = big.tile([128, 8, HIDDEN], F16, tag="wvo")
        nc.gpsimd.dma_start(out=wo_s, in_=woT_d.rearrange("(n p) m -> p n m", p=128))
        ctx0 = big.tile([128, 8, L], F16, tag="hT")
        ctx_t[0] = ctx0
        ctx1 = big.tile([128, 8, L], F16, tag="ctx0")
        ctx_t[1] = ctx1
        for bi in range(BLOC):
            for hg in range(HEADS // GH):
                if bi == 0 and hg == 0:
                    p2_assemble(0, 0, oq0)
                    continue
                oq = p2_expand(bi, hg)
                p2_assemble(bi, hg, oq)
        for mt in range(NT):
            p3_tile(mt)

    nc.compile()
    return nc


_CACHE = {}


def _get_nc(with_bias, with_affine):
    key = (with_bias, with_affine)
    if key not in _CACHE:
        _CACHE[key] = _build(with_bias, with_affine)
    return _CACHE[key]


def _host_prep(inputs):
    hs = np.ascontiguousarray(np.asarray(inputs["hidden_states"], np.float32))
    mask = np.asarray(inputs["attention_mask"])
    rel = np.asarray(inputs["relative_embedding"], np.float32)
    wqk = np.asarray(inputs["wqk"], np.float32)
    bqk = np.asarray(inputs["bqk"], np.float32)
    wv = np.asarray(inputs["wv"], np.float32)
    bv = np.asarray(inputs["bv"], np.float32)
    wo = np.asarray(inputs["wo"], np.float32)
    bo = np.asarray(inputs["bo"], np.float32)
    ln_g = np.asarray(inputs["ln_g"], np.float32)
    ln_b = np.asarray(inputs["ln_b"], np.float32)

    assert np.all(bo == 0.0), "kernel relies on bo == 0 (softmax-in-LN cancellation)"

    with_bias = bool(np.any(bqk != 0) or np.any(bv != 0))
    with_affine = bool(np.any(ln_g != 1) or np.any(ln_b != 0))

    wqkT = np.ascontiguousarray(wqk.T).astype(np.float64)
    wqkT[:, :HIDDEN] *= SCALE
    wqkT = wqkT.astype(np.float16)
    bqk2 = bqk.astype(np.float64)
    bqk2[:HIDDEN] *= SCALE
    bqk2 = bqk2.astype(np.float16)
    wvT = np.ascontiguousarray(wv.T).astype(np.float16)
    woT = np.ascontiguousarray(wo.T).astype(np.float16)
    relT = np.zeros((HIDDEN, 64), np.float16)
    relT[:, :REL] = rel.T
    Gq, Gk = _make_G()

    shared = {"wqkT": wqkT, "wvT": wvT, "woT": woT, "relT": relT,
              "Gq": Gq, "Gk": Gk}
    if with_bias:
        shared["bqk2"] = bqk2.reshape(1, -1)
        shared["bv2"] = bv.astype(np.float16).reshape(1, -1)
        shared["ones_row"] = np.ones((1, NTOK), np.float16)
    if with_affine:
        shared["g_bcast"] = np.ascontiguousarray(
            np.broadcast_to(ln_g, (128, HIDDEN)))
        shared["b_bcast"] = np.ascontiguousarray(
            np.broadcast_to(ln_b, (128, HIDDEN)))

    in_maps = []
    for c in range(NCORES):
        m = dict(shared)
        hs_c = hs[:, 2 * c:2 * c + 2, :]
        m["hs_tok"] = np.ascontiguousarray(
            hs_c.transpose(1, 0, 2).reshape(NTOK, HIDDEN)).astype(np.float16)
        mb = np.zeros((128, BLOC * 4), np.float32)
        for bi in range(BLOC):
            mrow = np.asarray(mask[2 * c + bi, 0, 0, :])
            for t in range(4):
                mb[:, 4 * bi + t] = np.where(mrow[128 * t:128 * t + 128], -1e9, 0.0)
        m["maskbias"] = mb
        in_maps.append(m)
    return in_maps, with_bias, with_affine


def kernel(**inputs):
    in_maps, with_bias, with_affine = _host_prep(inputs)
    nc = _get_nc(with_bias, with_affine)
    res = bass_utils.run_bass_kernel_spmd(nc, in_maps, core_ids=list(range(NCORES)))
    out = np.zeros((L, B, HIDDEN), np.float32)
    for c in range(NCORES):
        y = res.results[c]["out_y"]  # (NTOK, HIDDEN) token-major
        for bi in range(BLOC):
            out[:, 2 * c + bi, :] = y[512 * bi:512 * bi + 512, :]
    return out


# revision 50
# speedup vs baseline: 1.0231x; 1.0231x over previous
"""Trainium2 Bass kernel for nn_Bert_10187662426159 (DeBERTa-style
disentangled-attention BERT layer, L=512 B=16 D=1024 H=16).

Sharding: data-parallel over B - core c handles batch entries {2c, 2c+1}.

Per-core pipeline (ST orientation: scores stored [key j on partitions,
query i on free dim]; matmul operands f16, PSUM accumulation f32):
  P1  LN1 (no affine, f16 input) -> h ; PE-transpose -> hT [feat, tok]
  P1b q/k proj (feat-major, q pre-scaled by 1/sqrt(3*64)), v proj
      (token-major, with a ones-column per head for softmax row sums),
      rel-pos proj qkposT [2048, 63]
  P2  per group of 2 heads (x2 batch entries = 16 groups):
        qpST/pkST [63, 512] rank-63 positional factors (per head)
        window expansion [128, 640] per 128-row tile via 0/1 G-matrices
        (512-wide + 128-wide matmuls into single-bank PSUM tiles,
        converted f32->f16 on ACT/DVE)
      skew via DRAM bounce, one half-bounce per head: ONE contiguous
        write of the head's 8 windows [128, 8, 640] to a dedicated DRAM
        scratch, then ONE read back whose partition stride is
        (8*640 - 1): partition p starts at flat p*(8*640-1) + 127, i.e.
        column (127 - p) of its own row - the exact per-row shift that
        realigns delta-space diagonals to absolute key positions.
        2 large DMAs replace 24 small strided ones per head pair.
      per head, per 128-row j-tile, scores assemble in one PSUM bank:
        c2c matmul + pk via identity-matmul accumulate + qp via four
        identity-rhs transpose-matmul accumulates; ONE ACT exp with the
        attention mask as a per-partition bias (-1e9) -> P (f16, no
        max-subtraction needed: scores are bounded, exp(-1e9) = 0)
        ctx: [v | 1]^T @ P accumulates context AND row sums in PSUM;
        1/sum broadcast to 64 partitions via a ones-outer-product
        matmul into PSUM; DVE multiply normalizes
  P3  y = ctxT^T @ woT, LN2 + affine.

The DeBERTa take_along_axis gathers are exact: bucket expansion is a 0/1
matmul in delta-space and the diagonal re-alignment (skew) is one exact
DRAM round-trip (per-partition offsets are outside SBUF AP
expressiveness, but DRAM is flat-addressed, so a read AP with partition
stride (row_stride - 1) applies a per-partition shift of -p exactly).

PSUM layout (8 banks): b1 = 3x1 bank (P1 transposes, projections, score
assembly), w5 = 3x1 bank (positional factors, 512-wide expansion, P3),
w1 = 2x1 bank (128-wide expansion tail). The Exp activation table is
touched at startup so its lazy load does not stall the first softmax.
"""
import contextlib
import math
import sys

import numpy as np

sys.path.insert(0, "/opt/trn_rl_repo")
sys.path.insert(0, "/opt/trn_rl_repo/concourse")

import concourse.mybir as mybir  # noqa: E402
import concourse.tile as tile  # noqa: E402
from concourse import bacc, bass, bass_utils  # noqa: E402
from concourse.masks import make_identity  # noqa: E402

F32 = mybir.dt.float32
F16 = mybir.dt.float16

HIDDEN, HEADS, HEAD = 1024, 16, 64
BUCKET, MAXPOS, REL = 32, 512, 63
L, B = 512, 16
EPS = 1e-7
SCALE = 1.0 / math.sqrt(3 * HEAD)
WIN = 640
NCORES = 8
BLOC = B // NCORES          # 2 batch entries per core
NTOK = L * BLOC             # 1024 tokens per core
NT = NTOK // 128            # 8 token tiles
GH = 2                      # heads per attention group
AF = mybir.ActivationFunctionType


def _bucket_fn(delta):
    r = np.asarray(delta)
    mid = BUCKET // 2
    abs_pos = np.where((r < mid) & (r > -mid), mid - 1,
                       np.minimum(np.abs(r), MAXPOS - 1))
    with np.errstate(divide="ignore"):
        log_pos = (np.ceil(np.log(abs_pos.astype(np.float64) / mid)
                           / math.log((MAXPOS - 1) / mid) * (mid - 1))
                   .astype(np.int64) + mid)
    bucket_pos = np.where(abs_pos <= mid, r, log_pos * np.sign(r))
    return (BUCKET - 1 + bucket_pos).astype(np.int64)


def _make_G():
    Gq, Gk = [], []
    for t in range(4):
        w0 = -127 - 128 * t
        c = np.arange(WIN)
        dq = np.clip(-(w0 + c), -511, 511)
        dk = np.clip(+(w0 + c), -511, 511)
        Gq.append(_bucket_fn(dq)[None, :] == np.arange(REL)[:, None])
        Gk.append(_bucket_fn(dk)[None, :] == np.arange(REL)[:, None])
    return (np.stack(Gq).transpose(1, 0, 2).astype(np.float16),
            np.stack(Gk).transpose(1, 0, 2).astype(np.float16))  # [63, 4, 640]


def _build(with_bias: bool, with_affine: bool):
    nc = bacc.Bacc("TRN2", debug=False, num_devices=NCORES)

    hs_d = nc.dram_tensor("hs_tok", (NTOK, HIDDEN), F16, kind="ExternalInput").ap()
    mb_d = nc.dram_tensor("maskbias", (128, BLOC * 4), F32, kind="ExternalInput").ap()
    wqkT_d = nc.dram_tensor("wqkT", (HIDDEN, 2 * HIDDEN), F16, kind="ExternalInput").ap()
    wvT_d = nc.dram_tensor("wvT", (HIDDEN, HIDDEN), F16, kind="ExternalInput").ap()
    woT_d = nc.dram_tensor("woT", (HIDDEN, HIDDEN), F16, kind="ExternalInput").ap()
    relT_d = nc.dram_tensor("relT", (HIDDEN, 64), F16, kind="ExternalInput").ap()
    gq_d = nc.dram_tensor("Gq", (REL, 4, WIN), F16, kind="ExternalInput").ap()
    gk_d = nc.dram_tensor("Gk", (REL, 4, WIN), F16, kind="ExternalInput").ap()
    if with_bias:
        bqk_d = nc.dram_tensor("bqk2", (1, 2 * HIDDEN), F16, kind="ExternalInput").ap()
        bv_d = nc.dram_tensor("bv2", (1, HIDDEN), F16, kind="ExternalInput").ap()
        ones_d = nc.dram_tensor("ones_row", (1, NTOK), F16, kind="ExternalInput").ap()
    if with_affine:
        g_d = nc.dram_tensor("g_bcast", (128, HIDDEN), F32, kind="ExternalInput").ap()
        b_d = nc.dram_tensor("b_bcast", (128, HIDDEN), F32, kind="ExternalInput").ap()
    out_d = nc.dram_tensor("out_y", (NTOK, HIDDEN), F32, kind="ExternalOutput").ap()
    # DRAM scratch for the skew bounce: one buffer per (group parity, head)
    # so no two in-flight half-bounces share a buffer (the diagonal read AP
    # spans the whole buffer byte-range, which would serialize them).
    skw_h = [nc.dram_tensor(f"skw{i}", (128, 8, WIN), F16, kind="Internal")
             for i in range(4)]

    with tile.TileContext(nc) as tc, contextlib.ExitStack() as ctx:
        consts = ctx.enter_context(tc.tile_pool(name="consts", bufs=1))
        wpool = ctx.enter_context(tc.tile_pool(name="wpool", bufs=2))
        xio = ctx.enter_context(tc.tile_pool(name="xio", bufs=2))
        stat = ctx.enter_context(tc.tile_pool(name="stat", bufs=4))
        big = ctx.enter_context(tc.tile_pool(name="big", bufs=1))
        atts = ctx.enter_context(tc.tile_pool(name="atts", bufs=2))
        attp = ctx.enter_context(tc.tile_pool(name="attp", bufs=2))
        # PSUM (8 banks): b1 = 3 x 1 bank (assembly/proj), w5 = 3 x 1 bank
        # (factors + 512-wide expansion), w1 = 2 x 1 bank (128-wide tail).
        psp = ctx.enter_context(tc.tile_pool(name="psp", bufs=3, space="PSUM"))
        pse = ctx.enter_context(tc.tile_pool(name="pse", bufs=3, space="PSUM"))
        ps1 = ctx.enter_context(tc.tile_pool(name="ps1", bufs=2, space="PSUM"))

        # ---------- constants ----------
        ident16 = consts.tile([128, 128], F16)
        make_identity(nc, ident16)

        eps_t = consts.tile([128, 1], F32)
        nc.vector.memset(eps_t, EPS)
        gq_s = consts.tile([REL, 4, WIN], F16)
        gk_s = consts.tile([REL, 4, WIN], F16)
        nc.gpsimd.dma_start(out=gq_s, in_=gq_d)
        nc.gpsimd.dma_start(out=gk_s, in_=gk_d)
        mb_s = consts.tile([128, BLOC * 4], F32)
        nc.gpsimd.dma_start(out=mb_s, in_=mb_d)
        relT_s = consts.tile([128, NT, 64], F16)
        nc.gpsimd.dma_start(out=relT_s, in_=relT_d.rearrange("(n p) r -> p n r", p=128))
        onecol = consts.tile([1, 64], F16)
        nc.vector.memset(onecol, 1.0)
        # touch Exp early so the activation-table load doesn't stall the
        # first softmax at the P1b->P2 boundary
        warm = consts.tile([1, 1], F32)
        nc.scalar.activation(out=warm, in_=eps_t[0:1, :], func=AF.Exp)
        if with_bias:
            bqk_s = consts.tile([1, 2 * HIDDEN], F16)
            bv_s = consts.tile([1, HIDDEN], F16)
            ones_s = consts.tile([1, NTOK], F16)
            nc.sync.dma_start(out=bqk_s, in_=bqk_d)
            nc.sync.dma_start(out=bv_s, in_=bv_d)
            nc.sync.dma_start(out=ones_s, in_=ones_d)
        if with_affine:
            g_s = consts.tile([128, HIDDEN], F32)
            b_s = consts.tile([128, HIDDEN], F32)
            nc.sync.dma_start(out=g_s, in_=g_d)
            nc.sync.dma_start(out=b_s, in_=b_d)

        def layernorm_stats(y):
            """-> (rstd, -mean*rstd) [128,1] tiles for ACT normalize."""
            st = stat.tile([128, 2, nc.vector.BN_STATS_DIM], F32, tag="st")
            mv = stat.tile([128, nc.vector.BN_AGGR_DIM], F32, tag="mv")
            yr = y.rearrange("p (s d) -> p s d", s=2)
            for s in range(2):
                nc.vector.bn_stats(out=st[:, s, :], in_=yr[:, s, :])
            nc.vector.bn_aggr(out=mv, in_=st)
            rstd = stat.tile([128, 1], F32, tag="rstd")
            nc.scalar.activation(out=rstd, in_=mv[:, 1:2], func=AF.Sqrt,
                                 bias=eps_t, scale=1.0)
            nc.vector.reciprocal(out=rstd, in_=rstd)
            nmr = stat.tile([128, 1], F32, tag="nmr")
            nc.vector.tensor_mul(nmr, mv[:, 0:1], rstd)
            nc.vector.tensor_scalar_mul(nmr, nmr, -1.0)
            return rstd, nmr

        # ---------- P1: LN1 + transpose ----------
        hT = big.tile([128, NT, NTOK], F16, tag="hT")  # [feat, tok]
        hs3 = hs_d.rearrange("(n p) d -> n p d", p=128)

        def p1_tile(tt):
            x = xio.tile([128, HIDDEN], F16, tag="xh")
            (nc.sync if tt % 2 == 0 else nc.scalar).dma_start(out=x, in_=hs3[tt])
            rstd, nmr = layernorm_stats(x)
            h = xio.tile([128, HIDDEN], F16, tag="hyo")
            nc.vector.tensor_scalar(out=h, in0=x, scalar1=rstd, scalar2=nmr,
                                    op0=mybir.AluOpType.mult,
                                    op1=mybir.AluOpType.add)
            for fq in range(2):
                ptr = psp.tile([128, 4, 128], F16, tag="b1")
                for fb in range(4):
                    nc.tensor.matmul(ptr[:, fb, :],
                                     h[:, 512 * fq + 128 * fb:512 * fq + 128 * fb + 128],
                                     ident16, is_transpose=True)
                if fq == 0:
                    nc.vector.tensor_copy(
                        out=hT[:, 4 * fq:4 * fq + 4, 128 * tt:128 * tt + 128], in_=ptr)
                else:
                    nc.scalar.copy(
                        out=hT[:, 4 * fq:4 * fq + 4, 128 * tt:128 * tt + 128], in_=ptr)

        # ---------- P1b/P2/P3 building blocks (emitted interleaved) ----------
        qT = big.tile([128, 8, NTOK], F16, tag="qT")
        kT = big.tile([128, 8, NTOK], F16, tag="kT")
        vtm = big.tile([128, NT, HEADS, HEAD + 1], F16, tag="v")
        nc.vector.memset(vtm[:, :, :, HEAD:HEAD + 1], 1.0)
        qkposT = big.tile([128, 16, REL], F16, tag="qkposT")
        wqk3 = wqkT_d.rearrange("(n p) m -> n p m", p=128)
        wv_s = big.tile([128, 8, HIDDEN], F16, tag="wvo")
        nc.sync.dma_start(out=wv_s, in_=wvT_d.rearrange("(n p) m -> p n m", p=128))
        ctx_t = [None, None]
        wo_s = None
        out3 = out_d.rearrange("(n p) d -> n p d", p=128)
        nmi = 0

        def proj_qk(mg):
            """q/k + rel-pos projection for one 128-wide output feature tile."""
            nonlocal nmi
            w_m = wpool.tile([128, 8, 128], F16, tag="wqk")
            (nc.sync if nmi % 2 == 0 else nc.scalar).dma_start(
                out=w_m,
                in_=wqk3[:, :, 128 * mg:128 * mg + 128].transpose([1, 0, 2]))
            nmi += 1
            for nn_ in range(2):
                ns = slice(512 * nn_, 512 * nn_ + 512)
                pq = psp.tile([128, 512], F32, tag="b1")
                for k in range(8):
                    nc.tensor.matmul(pq, w_m[:, k, :], hT[:, k, ns],
                                     start=(k == 0),
                                     stop=(k == 7 and not with_bias))
                if with_bias:
                    nc.tensor.matmul(pq, bqk_s[:, 128 * mg:128 * mg + 128],
                                     ones_s[:, ns], start=False, stop=True)
                dst = qT if mg < 8 else kT
                nc.vector.tensor_copy(out=dst[:, mg % 8, ns], in_=pq)
            pqp = psp.tile([128, 64], F32, tag="b1")
            for k in range(8):
                nc.tensor.matmul(pqp, w_m[:, k, :], relT_s[:, k, :],
                                 start=(k == 0), stop=(k == 7 and not with_bias))
            if with_bias:
                nc.tensor.matmul(pqp, bqk_s[:, 128 * mg:128 * mg + 128],
                                 onecol, start=False, stop=True)
            nc.vector.tensor_copy(out=qkposT[:, mg, :], in_=pqp[:, :REL])

        def proj_v(mt):
            """v projection (token-major, with ones column) for one token tile."""
            for nn_ in range(2):
                ns = slice(512 * nn_, 512 * nn_ + 512)
                pv = psp.tile([128, 512], F32, tag="b1")
                for k in range(8):
                    nc.tensor.matmul(pv, hT[:, k, 128 * mt:128 * mt + 128],
                                     wv_s[:, k, ns], start=(k == 0),
                                     stop=(k == 7 and not with_bias))
                if with_bias:
                    nc.tensor.matmul(pv, ones_s[:, 128 * mt:128 * mt + 128],
                                     bv_s[:, ns], start=False, stop=True)
                nc.vector.tensor_copy(
                    out=vtm[:, mt, 8 * nn_:8 * nn_ + 8, 0:HEAD],
                    in_=pv.rearrange("p (h d) -> p h d", d=HEAD))

        def p2_expand(bi, hg):
            """windows + skew bounce for heads [2hg, 2hg+1] of entry bi."""
            toks = slice(512 * bi, 512 * bi + 512)
            par = (bi * (HEADS // GH) + hg) % 2
            # Eqk[:, hi, side, t, :] = delta-space windows (f16)
            Eqk = atts.tile([128, GH, 2, 4, WIN], F16, tag="eqk")
            outQPK = atts.tile([128, GH, 2, 4, 512], F16, tag="oqpk")
            for hi in range(GH):
                hd = GH * hg + hi
                po = 64 * (hd % 2)
                pf = slice(po, po + 64)
                qTh = qT[pf, hd // 2, toks]
                kTh = kT[pf, hd // 2, toks]
                kposTh = qkposT[pf, 8 + hd // 2, :]
                qposTh = qkposT[pf, hd // 2, :]
                pqp = pse.tile([REL, 512], F32, tag="w5")
                nc.tensor.matmul(pqp, kposTh, qTh)
                qpST = attp.tile([REL, 512], F16, tag="qpST")
                nc.vector.tensor_copy(out=qpST, in_=pqp)
                pkp = pse.tile([REL, 512], F32, tag="w5")
                nc.tensor.matmul(pkp, qposTh, kTh)
                pkST = attp.tile([REL, 512], F16, tag="pkST")
                nc.vector.tensor_copy(out=pkST, in_=pkp)
                for t in range(4):
                    rs = slice(128 * t, 128 * t + 128)
                    p5q = pse.tile([128, 512], F32, tag="w5")
                    nc.tensor.matmul(p5q, qpST[:, rs], gq_s[:, t, :512])
                    p1q = ps1.tile([128, 128], F32, tag="w1")
                    nc.tensor.matmul(p1q, qpST[:, rs], gq_s[:, t, 512:])
                    p5k = pse.tile([128, 512], F32, tag="w5")
                    nc.tensor.matmul(p5k, pkST[:, rs], gk_s[:, t, :512])
                    p1k = ps1.tile([128, 128], F32, tag="w1")
                    nc.tensor.matmul(p1k, pkST[:, rs], gk_s[:, t, 512:])
                    nc.scalar.copy(out=Eqk[:, hi, 0, t, :512], in_=p5q)
                    nc.vector.tensor_copy(out=Eqk[:, hi, 0, t, 512:], in_=p1q)
                    nc.vector.tensor_copy(out=Eqk[:, hi, 1, t, :512], in_=p5k)
                    nc.scalar.copy(out=Eqk[:, hi, 1, t, 512:], in_=p1k)
                # skew half-bounce for this head: one contiguous write + one
                # diagonal-AP read. Read partition p starts at flat
                # p*(8*WIN-1) + 127, i.e. column (127 - p) of its own row:
                # the exact per-row shift that realigns diagonals.
                buf = skw_h[2 * par + hi]
                nc.sync.dma_start(out=buf.ap(), in_=Eqk[:, hi])
                diag = bass.AP(buf, 127,
                               [[8 * WIN - 1, 128], [WIN, 8], [1, 512]])
                nc.gpsimd.dma_start(out=outQPK[:, hi], in_=diag)
            return outQPK

        def p2_assemble(bi, hg, outQPK):
            """score assembly + softmax + context for one head group."""
            toks = slice(512 * bi, 512 * bi + 512)
            # per head: score assembly in PSUM, one exp, ctx
            for hi in range(GH):
                hd = GH * hg + hi
                po = 64 * (hd % 2)
                pf = slice(po, po + 64)
                qTh = qT[pf, hd // 2, toks]
                kTh = kT[pf, hd // 2, toks]
                pctx = psp.tile([65, 512], F32, tag="b1")
                for jt in range(4):
                    js = slice(128 * jt, 128 * jt + 128)
                    pst = psp.tile([128, 512], F32, tag="b1")
                    nc.tensor.matmul(pst, kTh[:, js], qTh,
                                     start=True, stop=False)
                    nc.tensor.matmul(pst, ident16, outQPK[:, hi, 1, jt, :],
                                     start=False, stop=False)
                    for it in range(4):
                        nc.tensor.matmul(
                            pst[:, 128 * it:128 * it + 128],
                            outQPK[:, hi, 0, it, js], ident16,
                            start=False, stop=(it == 3))
                    P = attp.tile([128, 512], F16, tag="P")
                    nc.scalar.activation(
                        out=P, in_=pst, func=AF.Exp,
                        bias=mb_s[:, 4 * bi + jt:4 * bi + jt + 1])
                    nc.tensor.matmul(pctx, vtm[:, 4 * bi + jt, hd, :],
                                     P, start=(jt == 0), stop=(jt == 3))
                rsum = attp.tile([1, 512], F16, tag="rsum")
                with nc.allow_low_precision(reason="1/softmax-sum in f16 is ample"):
                    nc.vector.reciprocal(out=rsum, in_=pctx[64:65, :])
                pbc = psp.tile([64, 512], F32, tag="b1")
                nc.tensor.matmul(pbc, onecol, rsum)
                rb = attp.tile([64, 512], F16, tag="P")
                nc.scalar.copy(out=rb, in_=pbc)
                nc.vector.tensor_mul(ctx_t[bi][pf, hd // 2, :], pctx[0:64, :], rb)

        def p3_tile(mt):
            """wo projection + LN2 for one 128-token output tile."""
            bi, mtb = mt // 4, mt % 4
            y = xio.tile([128, HIDDEN], F32, tag="xy")
            for nn_ in range(2):
                ns = slice(512 * nn_, 512 * nn_ + 512)
                py = pse.tile([128, 512], F32, tag="w5")
                for k in range(8):
                    nc.tensor.matmul(
                        py, ctx_t[bi][:, k, 128 * mtb:128 * mtb + 128],
                        wo_s[:, k, ns], start=(k == 0), stop=(k == 7))
                nc.scalar.copy(out=y[:, ns], in_=py)
            rstd, nmr = layernorm_stats(y)
            yo = xio.tile([128, HIDDEN], F32, tag="hyo")
            nc.vector.tensor_scalar(out=yo, in0=y, scalar1=rstd, scalar2=nmr,
                                    op0=mybir.AluOpType.mult,
                                    op1=mybir.AluOpType.add)
            if with_affine:
                nc.vector.tensor_mul(yo, yo, g_s)
                nc.vector.tensor_add(yo, yo, b_s)
            nc.sync.dma_start(out=out3[mt], in_=yo)

        # ---------- emission order: sequential phases, except (a) the v
        # projection for tokens 0-511 is emitted between the two P1 halves
        # (it only needs hT token tiles 0-3), filling PE while DVE runs the
        # remaining LN chains, and (b) the first attention group's expansion
        # is pulled into the P1b tail so its skew bounce hides under the
        # projection matmuls.
        for tt in range(4):
            p1_tile(tt)
        for mt in range(4):
            proj_v(mt)
        for tt in range(4, NT):
            p1_tile(tt)
        for mg in [v for p in range(8) for v in (p, p + 8)]:
            proj_qk(mg)
        for mt in range(4, 6):
            proj_v(mt)
        oq0 = p2_expand(0, 0)
        for mt in range(6, NT):
            proj_v(mt)
        # wo prefetch: reuses the wv slot once the v projection drains;
        # context reuses the hT slot (all hT readers are emitted above).
        wo_s = big.tile([128, 8, HIDDEN], F16, tag="wvo")
        nc.gpsimd.dma_start(out=wo_s, in_=woT_d.rearrange("(n p) m -> p n m", p=128))
        ctx0 = big.tile([128, 8, L], F16, tag="hT")
        ctx_t[0] = ctx0
        ctx1 = big.tile([128, 8, L], F16, tag="ctx0")
        ctx_t[1] = ctx1
        for bi in range(BLOC):
            for hg in range(HEADS // GH):
                if bi == 0 and hg == 0:
                    p2_assemble(0, 0, oq0)
                    continue
                oq = p2_expand(bi, hg)
                p2_assemble(bi, hg, oq)
        for mt in range(NT):
            p3_tile(mt)

    nc.compile()
    return nc


_CACHE = {}


def _get_nc(with_bias, with_affine):
    key = (with_bias, with_affine)
    if key not in _CACHE:
        _CACHE[key] = _build(with_bias, with_affine)
    return _CACHE[key]


def _host_prep(inputs):
    hs = np.ascontiguousarray(np.asarray(inputs["hidden_states"], np.float32))
    mask = np.asarray(inputs["attention_mask"])
    rel = np.asarray(inputs["relative_embedding"], np.float32)
    wqk = np.asarray(inputs["wqk"], np.float32)
    bqk = np.asarray(inputs["bqk"], np.float32)
    wv = np.asarray(inputs["wv"], np.float32)
    bv = np.asarray(inputs["bv"], np.float32)
    wo = np.asarray(inputs["wo"], np.float32)
    bo = np.asarray(inputs["bo"], np.float32)
    ln_g = np.asarray(inputs["ln_g"], np.float32)
    ln_b = np.asarray(inputs["ln_b"], np.float32)

    assert np.all(bo == 0.0), "kernel relies on bo == 0 (softmax-in-LN cancellation)"

    with_bias = bool(np.any(bqk != 0) or np.any(bv != 0))
    with_affine = bool(np.any(ln_g != 1) or np.any(ln_b != 0))

    wqkT = np.ascontiguousarray(wqk.T).astype(np.float64)
    wqkT[:, :HIDDEN] *= SCALE
    wqkT = wqkT.astype(np.float16)
    bqk2 = bqk.astype(np.float64)
    bqk2[:HIDDEN] *= SCALE
    bqk2 = bqk2.astype(np.float16)
    wvT = np.ascontiguousarray(wv.T).astype(np.float16)
    woT = np.ascontiguousarray(wo.T).astype(np.float16)
    relT = np.zeros((HIDDEN, 64), np.float16)
    relT[:, :REL] = rel.T
    Gq, Gk = _make_G()

    shared = {"wqkT": wqkT, "wvT": wvT, "woT": woT, "relT": relT,
              "Gq": Gq, "Gk": Gk}
    if with_bias:
        shared["bqk2"] = bqk2.reshape(1, -1)
        shared["bv2"] = bv.astype(np.float16).reshape(1, -1)
        shared["ones_row"] = np.ones((1, NTOK), np.float16)
    if with_affine:
        shared["g_bcast"] = np.ascontiguousarray(
            np.broadcast_to(ln_g, (128, HIDDEN)))
        shared["b_bcast"] = np.ascontiguousarray(
            np.broadcast_to(ln_b, (128, HIDDEN)))

    in_maps = []
    for c in range(NCORES):
        m = dict(shared)
        hs_c = hs[:, 2 * c:2 * c + 2, :]
        m["hs_tok"] = np.ascontiguousarray(
            hs_c.transpose(1, 0, 2).reshape(NTOK, HIDDEN)).astype(np.float16)
        mb = np.zeros((128, BLOC * 4), np.float32)
        for bi in range(BLOC):
            mrow = np.asarray(mask[2 * c + bi, 0, 0, :])
            for t in range(4):
                mb[:, 4 * bi + t] = np.where(mrow[128 * t:128 * t + 128], -1e9, 0.0)
        m["maskbias"] = mb
        in_maps.append(m)
    return in_maps, with_bias, with_affine


def kernel(**inputs):
    in_maps, with_bias, with_affine = _host_prep(inputs)
    nc = _get_nc(with_bias, with_affine)
    res = bass_utils.run_bass_kernel_spmd(nc, in_maps, core_ids=list(range(NCORES)))
    out = np.zeros((L, B, HIDDEN), np.float32)
    for c in range(NCORES):
        y = res.results[c]["out_y"]  # (NTOK, HIDDEN) token-major
        for bi in range(BLOC):
            out[:, 2 * c + bi, :] = y[512 * bi:512 * bi + 512, :]
    return out
